# revision 24
# baseline (speedup 1.0000x reference)
"""LoRA-MoE layer (base dense + top-2 routed rank-16 LoRA experts) on 8 TRN2 cores.

Strategy: data-parallel over tokens (8192 tokens -> 1024/core), all weights
replicated, zero collectives. Per-core fused Bass/Tile kernel, all bf16
matmuls (single-pass router: measured 16/8192 top-2 misroutes, end-to-end
rel err ~5e-3 vs the 2e-2 gate).

Schedule (PE in-order, so emission order is PE execution order):
  phase A (k-loop over 16 x-chunks, 6 matmuls each = PE-bound over the
  whole input-DMA window):
    logits^T[e,t] = R^T.xh    u^T[er,t] = A^T.xh     (router + LoRA-A)
    plus 2 "early" base output tiles (t0o0, t0o1) on 2 spare PSUM banks,
    so W's 8.4MB streams entirely under phase-A compute.
  Then: top-2 softmax chains on DVE/ACT (transposed [t,8] tiles), weights
  transposed back + expanded to [er,t] via one-hot matmul,
  us^T = u^T * W_big * 2.0 (SCALING) in bf16.
  phase B: remaining 30 (ti,ob) output tiles in groups of 4 PSUM banks,
  k outermost within a group; each tile gets a fused us^T.T @ Bc finisher
  (stop=True) and is DMA'd straight from PSUM to HBM (no staging copy).

DMA notes (each dma_start costs ~625ns on the shared HWDGE + bytes/360GBps):
  xh and A are host-packed into one [128, KT, NT+ER] tensor so each x chunk
  is a single 288KB descriptor-friendly transfer; W chunks are [128, 2048]
  bf16 (4KB/partition contiguous); outputs go out per (ti,ob) tile.
"""

import os
import sys

import numpy as np


def _ensure_concourse():
    try:
        import concourse  # noqa: F401
    except ImportError:
        for p in ("/opt/trn_rl_repo", os.path.expanduser("~/.axon_site/_ro/trn_rl_repo")):
            if os.path.isdir(p):
                sys.path.insert(0, p)
                break


_ensure_concourse()

import ml_dtypes  # noqa: E402
import concourse.bass as bass  # noqa: E402,F401
import concourse.tile as tile  # noqa: E402
from concourse import bacc, mybir  # noqa: E402

F32 = mybir.dt.float32
BF16 = mybir.dt.bfloat16
X_AX = mybir.AxisListType.X
ALU = mybir.AluOpType
ACT = mybir.ActivationFunctionType

N_CORES = 8
N_TOK = 8192          # total tokens (4 x 2048)
NT = N_TOK // N_CORES  # tokens per core = 1024
D = 2048
O = 2048
E = 8
R = 16
ER = E * R            # 128
KT = D // 128         # 16 contraction chunks
TI = NT // 128        # 8 token tiles
OBS = 4               # o blocks of 512
TBS = 2               # token blocks of 512
XW = E + ER + NT      # packed x-chunk width: [router | LoRA-A | tokens]
HD1 = E + ER + 512    # first half-chunk: router + LoRA-A + first token block
TOFF = E + ER         # token column offset

EARLY = [(0, 0), (0, 1)]  # output tiles fused into phase A
GRP = 4                   # tiles per PSUM group in phase B

_NC_CACHE = {}
LAST_RESULTS = None


def _emit_chain(nc, smallp, trL, w_tiles):
    """Top-2 softmax weight chain for one 128-token tile (DVE/ACT ops)."""
    L = smallp.tile([128, E], F32, name="L", tag="L")
    nc.scalar.copy(L[:], trL[:])
    m1 = smallp.tile([128, 1], F32, name="m1", tag="m1")
    nc.vector.reduce_max(m1[:], L[:], axis=X_AX)
    nm1 = smallp.tile([128, 1], F32, name="nm1", tag="nm1")
    nc.scalar.mul(nm1[:], m1[:], -1.0)
    # mask out the top-1 entry, then find the 2nd max
    msk = smallp.tile([128, E], F32, name="msk", tag="msk")
    nc.vector.tensor_scalar(msk[:], L[:], m1[:], -1e30, ALU.is_equal, ALU.mult)
    L2 = smallp.tile([128, E], F32, name="L2", tag="L2")
    nc.vector.tensor_tensor(L2[:], L[:], msk[:], ALU.add)
    m2 = smallp.tile([128, 1], F32, name="m2", tag="m2")
    nc.vector.reduce_max(m2[:], L2[:], axis=X_AX)
    eL = smallp.tile([128, E], F32, name="eL", tag="eL")
    nc.scalar.activation(eL[:], L[:], ACT.Exp, bias=nm1[:])
    ge = smallp.tile([128, E], F32, name="ge", tag="ge")
    nc.vector.tensor_scalar(ge[:], L[:], m2[:], None, ALU.is_ge)
    un = smallp.tile([128, E], F32, name="un", tag="un")
    nc.vector.tensor_tensor(un[:], eL[:], ge[:], ALU.mult)
    s = smallp.tile([128, 1], F32, name="s", tag="s")
    nc.vector.reduce_sum(s[:], un[:], axis=X_AX)
    r = smallp.tile([128, 1], F32, name="r", tag="r")
    nc.vector.reciprocal(r[:], s[:])
    r2 = smallp.tile([128, 1], F32, name="r2", tag="r2")
    nc.scalar.mul(r2[:], r[:], 2.0)  # fold SCALING = 2.0
    w = smallp.tile([128, E], F32, name="w", tag="w", bufs=8)
    nc.vector.tensor_scalar(w[:], un[:], r2[:], None, ALU.mult)
    w_tiles.append(w)


def _body(tc, nc, xa, WTb, Bc, Mm, Idn, out):
    with (
        tc.tile_pool(name="const", bufs=1) as constp,
        tc.tile_pool(name="small", bufs=4) as smallp,
        tc.tile_pool(name="stage", bufs=4) as stagep,
    ):
        # PSUM pools with manual lifetimes (8 banks total):
        #   phase A: trwb(1) + early(2) + p1(4), one spare
        #   groups 0-1: trwb(1) + early(2) + mainA(5): 7 banks absorb the
        #     W stream faster than it arrives (7x512 rows > 1.46us/chunk)
        #   groups 2+: mainB(8) double-buffered - no bank-reuse stalls
        # trL transposes and the wb expansions are strictly sequential
        # users, so they share a single rotating bank.
        pstrp = tc.alloc_tile_pool(name="ps_tr", bufs=1, space="PSUM")
        psearlyp = tc.alloc_tile_pool(name="ps_early", bufs=1, space="PSUM")
        # ---- resident SBUF tensors ----
        WTb_sb = constp.tile([128, KT, O], BF16, name="WTb_sb")
        xa_sb = constp.tile([128, KT, XW], BF16, name="xa_sb")
        Bc_sb = constp.tile([ER, O], BF16, name="Bc_sb")
        Mm_sb = constp.tile([E, ER], BF16, name="Mm_sb")
        Id_sb = constp.tile([128, 128], F32, name="Id_sb")
        lg_sb = constp.tile([E, NT], F32, name="lg_sb")
        wT_sb = constp.tile([E, NT], BF16, name="wT_sb")
        u_sb = constp.tile([ER, NT], F32, name="u_sb")
        us_sb = constp.tile([ER, NT], BF16, name="us_sb")

        def xh(k, sl):  # token columns of packed chunk k
            return xa_sb[:, k, TOFF + sl.start:TOFF + sl.stop]

        def at(k):      # LoRA-A columns of packed chunk k
            return xa_sb[:, k, E:TOFF]

        def rt(k):      # router columns of packed chunk k
            return xa_sb[:, k, :E]

        # DMA order (single HWDGE FIFO): router weights + packed x chunks
        # (Mm/Idn slipped in early - phase A is PE-bound so the x stream has
        # slack), then W, then Bc (first needed well after W's tail).
        for k in range(KT):
            if k == 0:
                # split chunk 0 so PE's first matmuls start ~1us earlier;
                # the first half carries R and A plus the first token block
                nc.sync.dma_start(xa_sb[:, 0, :HD1], xa[:, 0, :HD1])
                nc.sync.dma_start(xa_sb[:, 0, HD1:], xa[:, 0, HD1:])
            else:
                nc.sync.dma_start(xa_sb[:, k, :], xa[:, k, :])
        for k in range(KT):
            nc.sync.dma_start(WTb_sb[:, k, :], WTb[:, k, :])
            if k == 0:
                # Idn first needed by the trL transposes (~27us), Mm by the
                # one-hot expansion (~44us)
                nc.sync.dma_start(Mm_sb[:], Mm[:])
                nc.sync.dma_start(Id_sb[:], Idn[:])
        nc.sync.dma_start(Bc_sb[:], Bc[:])

        # ---- phase A: router + LoRA-A + 2 early output tiles ----
        early_ps = [
            psearlyp.tile([128, 512], F32, name=f"early{i}", tag=f"e{i}")
            for i in range(len(EARLY))
        ]
        with tc.tile_pool(name="ps_p1", bufs=1, space="PSUM") as psp1:
            lg_ps = [psp1.tile([E, 512], F32, name=f"lgps{tb}", tag=f"lg{tb}")
                     for tb in range(TBS)]
            u_ps = [psp1.tile([ER, 512], F32, name=f"ups{tb}", tag=f"u{tb}")
                    for tb in range(TBS)]
            for k in range(KT):
                st, sp = (k == 0), (k == KT - 1)
                # chunk 0 arrives as two halves: order its matmuls so the
                # first three only touch tokens 0-511 (first half)
                nc.tensor.matmul(lg_ps[0][:], rt(k), xh(k, slice(0, 512)),
                                 start=st, stop=sp)
                for i, (ti, ob) in enumerate(EARLY):
                    nc.tensor.matmul(
                        early_ps[i][:],
                        xh(k, slice(ti * 128, (ti + 1) * 128)),
                        WTb_sb[:, k, ob * 512:(ob + 1) * 512],
                        start=st, stop=False,
                    )
                nc.tensor.matmul(lg_ps[1][:], rt(k), xh(k, slice(512, 1024)),
                                 start=st, stop=sp)
                for tb in range(TBS):
                    sl = slice(tb * 512, (tb + 1) * 512)
                    nc.tensor.matmul(u_ps[tb][:], at(k), xh(k, sl),
                                     start=st, stop=sp)
            # evacuate PSUM so phase B's pool can take these banks
            # (lg on ACT, u on DVE: two parallel streams)
            for tb in range(TBS):
                sl = slice(tb * 512, (tb + 1) * 512)
                nc.scalar.copy(lg_sb[:, sl], lg_ps[tb][:])
                nc.vector.tensor_copy(u_sb[:, sl], u_ps[tb][:])

        # ---- routing chains (DVE/ACT) + phase B ----
        tiles = [(ti, ob) for ti in range(TI) for ob in range(OBS)
                 if (ti, ob) not in EARLY]
        # groups 0-1: 5 tiles (mainA).  mainB groups round-robin over 8
        # single-buffer tags so each group's banks were freed two groups
        # (>10us) earlier - no bank-reuse stalls; the first mainB group has
        # 3 tiles so it lands exactly on the earliest-freed (tr/early)
        # banks.  Last two groups are single tiles so the kernel tail is
        # one staging copy + one out-DMA latency chain.
        groups = [tiles[0:5], tiles[5:10], tiles[10:13]]
        groups += [tiles[i:min(i + GRP, len(tiles) - 2)]
                   for i in range(13, len(tiles) - 2, GRP)]
        groups += [[tiles[-2]], [tiles[-1]]]
        w_tiles = []

        def emit_wexpand(tb):
            # transpose per-token weights back + expand to [er, t] for the
            # token half tb; chains for its tiles are long done by now.
            for ti in range(tb * 4, tb * 4 + 4):
                sl = slice(ti * 128, (ti + 1) * 128)
                trW = pstrp.tile([E, 128], F32, name="trW", tag="trwb",
                                 padded_shape=[128, 512])
                nc.tensor.transpose(trW[:], w_tiles[ti][:], Id_sb[:])
                nc.scalar.copy(wT_sb[:, sl], trW[:])
            sl = slice(tb * 512, (tb + 1) * 512)
            wb_ps = pstrp.tile([ER, 512], F32, name="wbps", tag="trwb")
            nc.tensor.matmul(wb_ps[:], Mm_sb[:], wT_sb[:, sl],
                             start=True, stop=True)
            nc.vector.tensor_tensor(us_sb[:, sl], u_sb[:, sl],
                                    wb_ps[:], ALU.mult)

        fin_count = [0]

        def emit_finish(ps_tile, ti, ob):
            # fused LoRA-B finisher; stage PSUM->SBUF (alternating ACT/DVE,
            # DMA cannot read PSUM) and store
            osl = slice(ob * 512, (ob + 1) * 512)
            tsl = slice(ti * 128, (ti + 1) * 128)
            nc.tensor.matmul(ps_tile[:], us_sb[:, tsl], Bc_sb[:, osl],
                             start=False, stop=True)
            st = stagep.tile([128, 512], F32, name="st", tag="st", bufs=6)
            if fin_count[0] % 2 == 0:
                nc.scalar.copy(st[:], ps_tile[:])
            else:
                nc.vector.tensor_copy(st[:], ps_tile[:])
            fin_count[0] += 1
            nc.sync.dma_start(out[tsl, osl], st[:])

        tag_rr = [0]

        def emit_group(pool, gi, grp, ntags):
            pss = []
            for i in range(len(grp)):
                t = tag_rr[0] % ntags
                tag_rr[0] += 1
                pss.append(pool.tile([128, 512], F32, name=f"mm{gi}_{i}",
                                     tag=f"mm{t}", bufs=1))
            for k in range(KT):
                for i, (ti, ob) in enumerate(grp):
                    nc.tensor.matmul(
                        pss[i][:],
                        xh(k, slice(ti * 128, (ti + 1) * 128)),
                        WTb_sb[:, k, ob * 512:(ob + 1) * 512],
                        start=(k == 0), stop=False,
                    )
                if gi == 0 and 1 <= k <= TI:
                    # logits -> token-major [t, 8] tiles, one per k-step;
                    # each trL's evacuation (first chain op, ACT) gets a
                    # full chunk of matmul shadow before the single tr
                    # bank is reused.  Chains run on DVE/ACT underneath.
                    ti = k - 1
                    sl = slice(ti * 128, (ti + 1) * 128)
                    trL = pstrp.tile([128, E], F32, name="trL", tag="trwb",
                                     padded_shape=[128, 512])
                    nc.tensor.transpose(trL[:], lg_sb[:, sl], Id_sb[:E, :E])
                    _emit_chain(nc, smallp, trL, w_tiles)
                if gi == 1 and k == 8:
                    # mid-k-loop: its us multiply (DVE) lands well before
                    # the ps_tr bank is recycled for group 2
                    emit_wexpand(1)
            if gi == 0:
                emit_wexpand(0)
                for i, (ti, ob) in enumerate(EARLY):
                    emit_finish(early_ps[i], ti, ob)
            for i, (ti, ob) in enumerate(grp):
                emit_finish(pss[i], ti, ob)

        # groups 0-1 on a 5-bank pool while tr/early banks are still live
        psmainA = tc.alloc_tile_pool(name="ps_mainA", bufs=1, space="PSUM")
        for gi in (0, 1):
            emit_group(psmainA, gi, groups[gi], 5)
        # LIFO release of everything phase B no longer needs
        psmainA.release()
        psearlyp.release()
        pstrp.release()
        # all 8 banks free now: 8 rotating tags
        tag_rr[0] = 0
        psmainB = tc.alloc_tile_pool(name="ps_mainB", bufs=1, space="PSUM")
        for gi in range(2, len(groups)):
            emit_group(psmainB, gi, groups[gi], 8)
        psmainB.release()


def build_nc():
    nc = bacc.Bacc("TRN2", target_bir_lowering=False, debug=False, num_devices=N_CORES)
    xa = nc.dram_tensor("xa", [128, KT, XW], BF16, kind="ExternalInput").ap()
    WTb = nc.dram_tensor("WTb", [128, KT, O], BF16, kind="ExternalInput").ap()
    Bc = nc.dram_tensor("Bc", [ER, O], BF16, kind="ExternalInput").ap()
    Mm = nc.dram_tensor("Mm", [E, ER], BF16, kind="ExternalInput").ap()
    Idn = nc.dram_tensor("Idn", [128, 128], F32, kind="ExternalInput").ap()
    out = nc.dram_tensor("out", [NT, O], F32, kind="ExternalOutput").ap()
    with tile.TileContext(nc) as tc:
        _body(tc, nc, xa, WTb, Bc, Mm, Idn, out)
    nc.compile()
    return nc


def get_nc():
    if "nc" not in _NC_CACHE:
        _NC_CACHE["nc"] = build_nc()
    return _NC_CACHE["nc"]


def make_in_maps(x, weight, lora_A, lora_B, router_w):
    x = np.ascontiguousarray(np.asarray(x, dtype=np.float32)).reshape(N_TOK, D)
    weight = np.asarray(weight, dtype=np.float32)
    lora_A = np.asarray(lora_A, dtype=np.float32)
    lora_B = np.asarray(lora_B, dtype=np.float32)
    router_w = np.asarray(router_w, dtype=np.float32)

    def to_pk(a):
        # [D, C] -> [128, KT, C]: partition p holds row k*128+p for each k chunk
        return np.ascontiguousarray(a.reshape(KT, 128, a.shape[1]).transpose(1, 0, 2))

    WTbm = to_pk(weight.T).astype(ml_dtypes.bfloat16)
    ATm = to_pk(lora_A.reshape(ER, D).T).astype(ml_dtypes.bfloat16)
    RTm = to_pk(np.ascontiguousarray(router_w.T)).astype(ml_dtypes.bfloat16)
    Bcm = np.ascontiguousarray(lora_B.transpose(0, 2, 1).reshape(ER, O)).astype(ml_dtypes.bfloat16)
    Mmm = np.zeros((E, ER), dtype=np.float32)
    for e in range(E):
        Mmm[e, e * R:(e + 1) * R] = 1.0
    Mmm = Mmm.astype(ml_dtypes.bfloat16)
    Idn = np.eye(128, dtype=np.float32)

    in_maps = []
    for c in range(N_CORES):
        xTc = to_pk(np.ascontiguousarray(x[c * NT:(c + 1) * NT].T)).astype(
            ml_dtypes.bfloat16)
        xac = np.concatenate([RTm, ATm, xTc], axis=2)  # [128, KT, E+ER+NT]
        in_maps.append({
            "xa": np.ascontiguousarray(xac),
            "WTb": WTbm,
            "Bc": Bcm,
            "Mm": Mmm,
            "Idn": Idn,
        })
    return in_maps


def kernel(x, weight, lora_A, lora_B, router_w):
    global LAST_RESULTS
    from concourse.bass_utils import run_bass_kernel_spmd

    in_maps = make_in_maps(x, weight, lora_A, lora_B, router_w)
    nc = get_nc()
    trace = bool(os.environ.get("KBENCH_TRACE"))
    res = run_bass_kernel_spmd(nc, in_maps, core_ids=list(range(N_CORES)), trace=trace)
    LAST_RESULTS = res
    outs = [np.asarray(res.results[c]["out"], dtype=np.float32) for c in range(N_CORES)]
    return np.concatenate(outs, axis=0).reshape(4, 2048, 2048)
